# revision 33
# baseline (speedup 1.0000x reference)
"""Trainium2 Bass kernel for nn_RNN_61125974557431.

Keras-style LSTM (gate order [i,f,g,o], recurrent_activation=sigmoid,
activation=relu on candidate AND cell in h), followed by a small dense head:
    xz = x @ Wx + b                      # precomputed per step on PE (proj)
    per t: z = xz_t + h @ Wh
           i,f,o = sigmoid(z_...), g = relu(z_g)
           c = f*c + i*g                 # c >= 0 inductively => relu(c) == c
           h = o * c
    y = sigmoid(relu(relu(h@W2+b2)@W3+b3)@Wo+bo)

Sharding: pure data parallel over batch, 8 cores x 256 rows.
Per-core layout: 2 independent chains of 128 batch (latency hiding), each
chain packs G=2 groups of 64 batch on the partition axis:
  psum z [128 units, 64 batch]: unit blocks [i(0:32)|f(32:64)|o(64:96)|g(96:128)],
  each block = [group0(16); group1(16)] h-units.
Engine plan per chain-step:
  PE : rec matmul (blockdiag Wh), proj matmul every 8 steps (N=512 burst)
  ACT: sigmoid(i,f) psum->sbuf, relu(g) psum->sbuf@0, sigmoid(o) psum->psum
  DVE: m2 = i*g, c = m1+m2 (in place), h = c*o (mixed sbuf+psum)
  GP : m1 = f*c (all-sbuf, same base partition 32)
"""
import os
import sys

sys.path.insert(0, "/opt/trn_rl_repo")

import numpy as np

B, T, F, H = 2048, 512, 64, 16
NCORES = 8
BC = B // NCORES            # 256 batch per core
NCHAIN = 2                  # independent chains per core
CB = BC // NCHAIN           # 128 batch per chain
G = 2                       # groups packed on partitions per chain
BG = CB // G                # 64 batch per group (= free dim of most ops)
PROJ_STEPS = 8              # steps per proj burst: 8 * BG = 512 = 1 psum bank
TCHUNK = 32                 # steps per x DMA chunk
M1_ENGINE = os.environ.get("LSTM_M1_ENGINE", "vector")   # 'gpsimd' | 'vector'
T_RUN = int(os.environ.get("LSTM_T", T))                 # debug: fewer steps

# Keras kernel column order is [i, f, g, o] blocks of H.
_KERAS_GATE = {0: 0, 1: 1, 2: 3, 3: 2}   # our block (i,f,o,g) -> keras block


def _unit_perm():
    """keras column index for each of our 128 output units."""
    cols = np.zeros(4 * H * G, dtype=np.int64)
    grp = np.zeros(4 * H * G, dtype=np.int64)
    for u in range(4 * H * G):
        blk = u // (H * G)              # 0=i 1=f 2=o 3=g
        within = u % (H * G)
        g = within // H
        hh = within % H
        cols[u] = _KERAS_GATE[blk] * H + hh
        grp[u] = g
    return cols, grp


def _pack_weights(Wx, Wh, b, W2, b2, W3, b3, Wo, bo):
    cols, grp = _unit_perm()
    U = 4 * H * G
    wh2 = np.zeros((H * G, U), dtype=np.float32)
    wx2 = np.zeros((F * G, U), dtype=np.float32)
    for u in range(U):
        gg = grp[u]
        wh2[H * gg:H * (gg + 1), u] = Wh[:, cols[u]]
        wx2[F * gg:F * (gg + 1), u] = Wx[:, cols[u]]
    ball = b[cols].astype(np.float32).reshape(U, 1)

    w2d = np.zeros((H * G, 8 * G), dtype=np.float32)
    w3d = np.zeros((8 * G, 4 * G), dtype=np.float32)
    wod = np.zeros((4 * G, G), dtype=np.float32)
    for g in range(G):
        w2d[H * g:H * (g + 1), 8 * g:8 * (g + 1)] = W2
        w3d[8 * g:8 * (g + 1), 4 * g:4 * (g + 1)] = W3
        wod[4 * g:4 * (g + 1), g:g + 1] = Wo
    b2d = np.tile(b2, G).astype(np.float32).reshape(8 * G, 1)
    b3d = np.tile(b3, G).astype(np.float32).reshape(4 * G, 1)
    bod = np.tile(bo, G).astype(np.float32).reshape(G, 1)
    return wh2, wx2, ball, w2d, b2d, w3d, b3d, wod, bod


def _build_module():
    from contextlib import ExitStack

    import concourse.bass as bass
    import concourse.tile as tile
    from concourse import mybir
    from concourse.tile_rust import add_dep_helper

    AF = mybir.ActivationFunctionType
    nc = bass.Bass("TRN2", debug=False)

    # DRAM tensors (per-core inputs supplied via in_maps)
    xs = [
        nc.dram_tensor(f"xs{c}", [F * G, T * BG], mybir.dt.float32,
                       kind="ExternalInput").ap()
        for c in range(NCHAIN)
    ]
    wh2_d = nc.dram_tensor("wh2", [H * G, 128], mybir.dt.float32,
                           kind="ExternalInput").ap()
    wx2_d = nc.dram_tensor("wx2", [F * G, 128], mybir.dt.float32,
                           kind="ExternalInput").ap()
    ball_d = nc.dram_tensor("ball", [128, 1], mybir.dt.float32,
                            kind="ExternalInput").ap()
    w2d_d = nc.dram_tensor("w2d", [H * G, 8 * G], mybir.dt.float32,
                           kind="ExternalInput").ap()
    b2d_d = nc.dram_tensor("b2d", [8 * G, 1], mybir.dt.float32,
                           kind="ExternalInput").ap()
    w3d_d = nc.dram_tensor("w3d", [8 * G, 4 * G], mybir.dt.float32,
                           kind="ExternalInput").ap()
    b3d_d = nc.dram_tensor("b3d", [4 * G, 1], mybir.dt.float32,
                           kind="ExternalInput").ap()
    wod_d = nc.dram_tensor("wod", [4 * G, G], mybir.dt.float32,
                           kind="ExternalInput").ap()
    bod_d = nc.dram_tensor("bod", [G, 1], mybir.dt.float32,
                           kind="ExternalInput").ap()
    y_d = nc.dram_tensor("y", [2 * NCHAIN, BG], mybir.dt.float32,
                         kind="ExternalOutput").ap()

    n_chunks = (T_RUN + TCHUNK - 1) // TCHUNK

    with tile.TileContext(nc) as tc, ExitStack() as ctx:
        # All tiles are STATIC (bufs=1, allocated once, rotated manually):
        # pool-slot releases inject un-strippable waits, and hardware
        # instructions only carry ONE sync-wait slot.
        const = ctx.enter_context(tc.tile_pool(name="const", bufs=1))
        state = ctx.enter_context(tc.tile_pool(name="state", bufs=1))
        pspool = ctx.enter_context(tc.tile_pool(name="ps", bufs=1, space="PSUM"))

        # --- wait-count management --------------------------------------
        all_insts = {}          # name -> BassInstruction

        def reg(bi):
            all_insts[bi.ins.name] = bi
            # Same-engine sync deps -> nosync ordering edges: engines
            # complete strictly in order, so program order suffices and the
            # single hardware wait slot stays free for cross-engine deps.
            eng = bi.ins.engine
            for nm in list(bi.ins.sync_dependency_names()):
                di = all_insts.get(nm)
                if di is not None and di.ins.engine == eng:
                    bi.ins.try_remove_dependency(nm)
                    add_dep_helper(bi.ins, di.ins, sync=False,
                                   reason="same-engine order")
            return bi

        def absorb(op, nop_maker, pred):
            """Move op's sync deps matching pred onto same-engine sequencer
            nops (one per dependency proc, each carrying a single-sem wait),
            ordered before op via nosync edges; drop those deps from op."""
            groups = {}
            for nm in list(op.ins.sync_dependency_names()):
                di = all_insts.get(nm)
                if di is not None and pred(di):
                    groups.setdefault(str(di.ins.engine), []).append((nm, di))
            if not groups:
                return False
            prev = None
            for _eng, deps in sorted(groups.items()):
                n = nop_maker()
                for nm, di in deps:
                    add_dep_helper(n.ins, di.ins, reason="absorbed wait")
                    op.ins.try_remove_dependency(nm)
                if prev is not None:
                    add_dep_helper(n.ins, prev.ins, sync=False,
                                   reason="chain wait nops")
                prev = n
            add_dep_helper(op.ins, prev.ins, sync=False,
                           reason="order after wait-absorbing nop")
            return True

        def is_act(bi):
            return bi.ins.engine == mybir.EngineType.Activation

        def is_dve(bi):
            return bi.ins.engine == mybir.EngineType.DVE

        def is_pe(bi):
            return bi.ins.engine == mybir.EngineType.PE

        def is_dve_init(bi):
            # fence copies / state memsets — safe to absorb off matmuls;
            # never matches the hmul data dependency (TensorTensor).
            return is_dve(bi) and bi.ins.opcode in ("TensorCopy", "Memset")

        tail_deps = []

        def load_const(name, shape, dram_ap):
            raw = const.tile(shape, mybir.dt.float32, name=f"{name}_raw")
            tail_deps.append(reg(nc.gpsimd.dma_start(raw[:], dram_ap[:])))
            t = const.tile(shape, mybir.dt.float32, name=name)
            reg(nc.vector.tensor_copy(t[:], raw[:]))
            return t

        wh2_t = load_const("wh2t", [H * G, 128], wh2_d)
        wx2_t = load_const("wx2t", [F * G, 128], wx2_d)
        ball_t = load_const("ballt", [128, 1], ball_d)
        w2d_t = load_const("w2dt", [H * G, 8 * G], w2d_d)
        b2d_t = load_const("b2dt", [8 * G, 1], b2d_d)
        w3d_t = load_const("w3dt", [8 * G, 4 * G], w3d_d)
        b3d_t = load_const("b3dt", [4 * G, 1], b3d_d)
        wod_t = load_const("wodt", [4 * G, G], wod_d)
        bod_t = load_const("bodt", [G, 1], bod_d)

        # persistent state per chain; c lives in rows 32:64 of a 64-row tile
        # so its base partition matches the f block of the sigmoid output.
        h_t, cfull_t, o_ps, sig_t, g_t, m1_t, m2_t = [], [], [], [], [], [], []
        zb_t, xt_t = [], []
        for c in range(NCHAIN):
            ht = state.tile([H * G, BG], mybir.dt.float32, name=f"h{c}")
            reg(nc.vector.memset(ht[:], 0.0))
            h_t.append(ht)
            cf = state.tile([2 * H * G, BG], mybir.dt.float32, name=f"c{c}")
            reg(nc.vector.memset(cf[32:64, :], 0.0))
            cfull_t.append(cf)
            o_ps.append(pspool.tile([128, 8 * BG], mybir.dt.float32,
                                    name=f"ob{c}"))
            sig_t.append(state.tile([2 * H * G, BG], mybir.dt.float32,
                                    name=f"sig{c}"))
            g_t.append(state.tile([H * G, BG], mybir.dt.float32,
                                  name=f"g{c}"))
            m1_t.append(state.tile([2 * H * G, BG], mybir.dt.float32,
                                   name=f"m1{c}"))
            m2_t.append(state.tile([2 * H * G, BG], mybir.dt.float32,
                                   name=f"m2{c}"))
            zb_t.append([pspool.tile([128, PROJ_STEPS * BG], mybir.dt.float32,
                                     name=f"zb{c}_{k}") for k in range(2)])
            xt_t.append([state.tile([F * G, TCHUNK * BG], mybir.dt.float32,
                                    name=f"xt{c}_{k}") for k in range(2)])

        m1_eng = nc.gpsimd if M1_ENGINE == "gpsimd" else nc.vector

        def dma_chunk(c, j):
            tl = xt_t[c][j % 2]
            d = reg(nc.gpsimd.dma_start(
                tl[:], xs[c][:, j * TCHUNK * BG:(j + 1) * TCHUNK * BG]))
            # keep only the queue-serialization wait on the DMA itself;
            # the WAR dep on the proj matmuls reading the old chunk goes
            # through a POOL sequencer nop.
            absorb(d, nc.gpsimd.nop, is_pe)
            tail_deps.append(d)
            return tl

        for c in range(NCHAIN):
            dma_chunk(c, 0)

        for t in range(T_RUN):
            slot = t % PROJ_STEPS
            chunk = t // TCHUNK
            for c in range(NCHAIN):
                if t % TCHUNK == TCHUNK // 2 and chunk + 1 < n_chunks:
                    dma_chunk(c, chunk + 1)
                def _mm_fixup(mm):
                    # matmuls keep their DVE(hmul)/DMA wait; ACT deps (z-bank
                    # reader history) and init copies go through nops.
                    absorb(mm, nc.tensor.nop,
                           lambda di: is_act(di) or is_dve_init(di))

                if slot == 0:
                    # new proj burst into the alternate z bank
                    z = zb_t[c][(t // PROJ_STEPS) % 2]
                    zt_cur = z
                    off = (t % TCHUNK) * BG
                    mm_proj = reg(nc.tensor.matmul(
                        z[:], wx2_t[:],
                        xt_t[c][chunk % 2][:, off:off + PROJ_STEPS * BG],
                        start=True, stop=True,
                    ))
                    _mm_fixup(mm_proj)
                z = zb_t[c][(t // PROJ_STEPS) % 2]
                sl = slice(slot * BG, (slot + 1) * BG)
                # recurrent matmul accumulates into this step's slot
                mm_rec = reg(nc.tensor.matmul(
                    z[:, sl], wh2_t[:], h_t[c][:],
                    start=False, stop=True, skip_group_check=True,
                ))
                _mm_fixup(mm_rec)
                sig, g_sb, m1, m2 = sig_t[c], g_t[c], m1_t[c], m2_t[c]
                # ACT ops keep their PE wait; DVE-side deps (work-tile WAR,
                # o-slot WAR) go through ACT sequencer nops.
                # i,f sigmoid (+bias), psum -> sbuf rows 0:64
                a_sig1 = reg(nc.scalar.activation(sig[0:64, :], z[0:64, sl],
                                                  AF.Sigmoid,
                                                  bias=ball_t[0:64, :]))
                absorb(a_sig1, nc.scalar.nop, is_dve)
                # g relu, psum rows 96:128 -> sbuf rows 0:32
                a_relu = reg(nc.scalar.activation(g_sb[:], z[96:128, sl],
                                                  AF.Relu,
                                                  bias=ball_t[96:128, :]))
                absorb(a_relu, nc.scalar.nop, is_dve)
                # o sigmoid, psum rows 64:96 -> o-bank psum rows 64:96
                osl = slice((t % 8) * BG, (t % 8 + 1) * BG)
                a_sig2 = reg(nc.scalar.activation(
                    o_ps[c][64:96, osl], z[64:96, sl],
                    AF.Sigmoid, bias=ball_t[64:96, :]))
                absorb(a_sig2, nc.scalar.nop, is_dve)
                # m2 = i * g  (both sbuf base 0) -> rows 32:64
                reg(nc.vector.tensor_mul(m2[32:64, :], sig[0:32, :], g_sb[:]))
                # m1 = f * c  (all sbuf base 32)
                reg(m1_eng.tensor_mul(m1[32:64, :], sig[32:64, :],
                                      cfull_t[c][32:64, :]))
                # c = m1 + m2 (in place)
                reg(nc.vector.tensor_add(cfull_t[c][32:64, :], m1[32:64, :],
                                         m2[32:64, :]))
                # h = c * o  (sbuf base 32 x psum base 64 -> sbuf base 0).
                # Its WAR dep on rec's h read is transitively covered via
                # sig2(t) <- rec(t); drop PE deps outright.
                hm = reg(nc.vector.tensor_mul(h_t[c][:], cfull_t[c][32:64, :],
                                              o_ps[c][64:96, osl]))
                for nm in list(hm.ins.sync_dependency_names()):
                    di = all_insts.get(nm)
                    if di is not None and is_pe(di):
                        hm.ins.try_remove_dependency(nm)

        # dense head per chain
        hp = pspool.tile([128, 4 * BG], mybir.dt.float32, name="headp")
        for c in range(NCHAIN):
            mmh1 = reg(nc.tensor.matmul(hp[0:8 * G, c * 2 * BG:c * 2 * BG + BG],
                                        w2d_t[:], h_t[c][:],
                                        start=True, stop=True))
            absorb(mmh1, nc.tensor.nop, lambda di: is_act(di) or is_dve_init(di))
            x2 = state.tile([8 * G, BG], mybir.dt.float32, name=f"x2{c}")
            a_x2 = reg(nc.scalar.activation(
                x2[:], hp[0:8 * G, c * 2 * BG:c * 2 * BG + BG],
                AF.Relu, bias=b2d_t[:]))
            absorb(a_x2, nc.scalar.nop, is_dve)
            mmh2 = reg(nc.tensor.matmul(
                hp[0:4 * G, c * 2 * BG + BG:c * 2 * BG + 2 * BG], w3d_t[:],
                x2[:], start=True, stop=True))
            absorb(mmh2, nc.tensor.nop, lambda di: is_act(di) or is_dve_init(di))
            x3 = state.tile([4 * G, BG], mybir.dt.float32, name=f"x3{c}")
            a_x3 = reg(nc.scalar.activation(
                x3[:], hp[0:4 * G, c * 2 * BG + BG:c * 2 * BG + 2 * BG],
                AF.Relu, bias=b3d_t[:]))
            absorb(a_x3, nc.scalar.nop, is_dve)
            ozp = o_ps[c]
            mmh3 = reg(nc.tensor.matmul(ozp[0:G, 0:BG], wod_t[:], x3[:],
                                        start=True, stop=True))
            absorb(mmh3, nc.tensor.nop,
                   lambda di: is_act(di) or is_dve(di))
            ych = state.tile([G, BG], mybir.dt.float32, name=f"y{c}")
            a_y = reg(nc.scalar.activation(ych[:], ozp[0:G, 0:BG], AF.Sigmoid,
                                           bias=bod_t[:]))
            absorb(a_y, nc.scalar.nop, is_dve)
            dma_y = reg(nc.gpsimd.dma_start(y_d[G * c:G * (c + 1), :], ych[:]))
            absorb(dma_y, nc.gpsimd.nop, is_act)
            tail_deps.append(mmh3)
            tail_deps.append(dma_y)
            tail_deps.append(a_y)

        # Tail: single-wait SP nops make the SP sequencer observe every
        # proc's final tick so the framework's kernel-tail Drain (one wait
        # slot) needs no further waits.
        prev = None
        for bi in tail_deps:
            n = nc.sync.nop()
            add_dep_helper(n.ins, bi.ins, reason="tail observe")
            if prev is not None:
                add_dep_helper(n.ins, prev.ins, sync=False,
                               reason="tail chain")
            prev = n

    return nc


_CACHE = {}


def _prep_x(x):
    """Per (core, chain): [F*G, T*BG] with rows 0:F = group0 (features-major),
    rows F:2F = group1; free layout (t, b)."""
    out = []
    for core in range(NCORES):
        chains = []
        for ch in range(NCHAIN):
            base = core * BC + ch * CB
            g0 = x[base:base + BG]                 # [BG, T, F]
            g1 = x[base + BG:base + 2 * BG]
            s0 = np.ascontiguousarray(g0.transpose(2, 1, 0)).reshape(F, T * BG)
            s1 = np.ascontiguousarray(g1.transpose(2, 1, 0)).reshape(F, T * BG)
            chains.append(np.concatenate([s0, s1], axis=0))
        out.append(chains)
    return out


def kernel(x, Wx, Wh, b, W2, b2, W3, b3, Wo, bo, _trace=False):
    from concourse import bass_utils

    x = np.asarray(x, dtype=np.float32)
    wh2, wx2, ball, w2d, b2d, w3d, b3d, wod, bod = _pack_weights(
        np.asarray(Wx, np.float32), np.asarray(Wh, np.float32),
        np.asarray(b, np.float32), np.asarray(W2, np.float32),
        np.asarray(b2, np.float32), np.asarray(W3, np.float32),
        np.asarray(b3, np.float32), np.asarray(Wo, np.float32),
        np.asarray(bo, np.float32))

    if "nc" not in _CACHE:
        _CACHE["nc"] = _build_module()
    nc = _CACHE["nc"]

    xprep = _prep_x(x)
    in_maps = []
    for core in range(NCORES):
        m = {
            "wh2": wh2, "wx2": wx2, "ball": ball,
            "w2d": w2d, "b2d": b2d, "w3d": w3d, "b3d": b3d,
            "wod": wod, "bod": bod,
        }
        for ch in range(NCHAIN):
            m[f"xs{ch}"] = xprep[core][ch]
        in_maps.append(m)

    res = bass_utils.run_bass_kernel_spmd(
        nc, in_maps, core_ids=list(range(NCORES)), trace=_trace,
    )
    ys = []
    for core in range(NCORES):
        y = res.results[core]["y"]          # [2*NCHAIN, BG]
        ys.append(y.reshape(BC))
    out = np.concatenate(ys).reshape(B, 1).astype(np.float32)
    if _trace:
        out = (out, res)
    return out


def bench(inputs, iters=20):
    """Median wall time of repeated device executes with device-resident
    inputs (upper bound on per-launch HW time; includes dispatch overhead)."""
    import time

    import jax
    import numpy as np_
    from jax.sharding import Mesh, NamedSharding, PartitionSpec

    from concourse import bass2jax, mybir

    if "nc" not in _CACHE:
        _CACHE["nc"] = _build_module()
    nc = _CACHE["nc"]

    x = np.asarray(inputs["x"], dtype=np.float32)
    wh2, wx2, ball, w2d, b2d, w3d, b3d, wod, bod = _pack_weights(
        *[np.asarray(inputs[k], np.float32) for k in
          ("Wx", "Wh", "b", "W2", "b2", "W3", "b3", "Wo", "bo")])
    xprep = _prep_x(x)
    in_maps = []
    for core in range(NCORES):
        m = {"wh2": wh2, "wx2": wx2, "ball": ball, "w2d": w2d, "b2d": b2d,
             "w3d": w3d, "b3d": b3d, "wod": wod, "bod": bod}
        for ch in range(NCHAIN):
            m[f"xs{ch}"] = xprep[core][ch]
        in_maps.append(m)

    bass2jax.install_neuronx_cc_hook()
    from jax.experimental.shard_map import shard_map

    partition_name = (nc.partition_id_tensor.name
                      if nc.partition_id_tensor else None)
    in_names, out_names, out_avals, zero_outs = [], [], [], []
    for alloc in nc.m.functions[0].allocations:
        if not isinstance(alloc, mybir.MemoryLocationSet):
            continue
        name = alloc.memorylocations[0].name
        if alloc.kind == "ExternalInput":
            if name != partition_name:
                in_names.append(name)
        elif alloc.kind == "ExternalOutput":
            out_names.append(name)
            shape = tuple(alloc.tensor_shape)
            np_dt = mybir.dt.np(alloc.dtype)
            out_avals.append(jax.core.ShapedArray(shape, np_dt))
            zero_outs.append(np_.zeros(shape, np_dt))
    n_params = len(in_names)
    n_outs = len(out_avals)
    all_names = in_names + out_names
    if partition_name is not None:
        all_names = all_names + [partition_name]

    def _body(*args):
        operands = list(args)
        if partition_name is not None:
            operands.append(bass2jax.partition_id_tensor())
        outs = bass2jax._bass_exec_p.bind(
            *operands,
            out_avals=tuple(out_avals),
            in_names=tuple(all_names),
            out_names=tuple(out_names),
            lowering_input_output_aliases=(),
            sim_require_finite=True,
            sim_require_nnan=True,
            nc=nc,
        )
        return tuple(outs)

    devices = jax.devices()[:NCORES]
    mesh = Mesh(np_.asarray(devices), ("core",))
    in_specs = (PartitionSpec("core"),) * (n_params + n_outs)
    out_specs = (PartitionSpec("core"),) * n_outs
    sharded = jax.jit(
        shard_map(_body, mesh=mesh, in_specs=in_specs, out_specs=out_specs,
                  check_rep=False),
        keep_unused=True,
    )
    shard = NamedSharding(mesh, PartitionSpec("core"))
    concat_in = [
        jax.device_put(
            np_.concatenate([np_.asarray(in_maps[c][nm]) for c in range(NCORES)],
                            axis=0), shard)
        for nm in in_names[:n_params]
    ]
    concat_zeros = [
        jax.device_put(np_.zeros((NCORES * z.shape[0], *z.shape[1:]), z.dtype),
                       shard)
        for z in zero_outs
    ]
    # warmup (compile)
    out = sharded(*concat_in, *concat_zeros)
    jax.block_until_ready(out)
    times = []
    for _ in range(iters):
        t0 = time.perf_counter()
        out = sharded(*concat_in, *concat_zeros)
        jax.block_until_ready(out)
        times.append(time.perf_counter() - t0)
    times.sort()
    return times[len(times) // 4] * 1e9   # lower-quartile wall ns


# revision 34
# speedup vs baseline: 1.0037x; 1.0037x over previous
"""Trainium2 Bass kernel for nn_RNN_61125974557431.

Keras-style LSTM (gate order [i,f,g,o], recurrent_activation=sigmoid,
activation=relu on candidate AND cell in h), followed by a small dense head:
    xz = x @ Wx + b                      # precomputed per step on PE (proj)
    per t: z = xz_t + h @ Wh
           i,f,o = sigmoid(z_...), g = relu(z_g)
           c = f*c + i*g                 # c >= 0 inductively => relu(c) == c
           h = o * c
    y = sigmoid(relu(relu(h@W2+b2)@W3+b3)@Wo+bo)

Sharding: pure data parallel over batch, 8 cores x 256 rows.
Per-core layout: 2 independent chains of 128 batch (latency hiding), each
chain packs G=2 groups of 64 batch on the partition axis:
  psum z [128 units, 64 batch]: unit blocks [i(0:32)|f(32:64)|o(64:96)|g(96:128)],
  each block = [group0(16); group1(16)] h-units.
Engine plan per chain-step:
  PE : rec matmul (blockdiag Wh), proj matmul every 8 steps (N=512 burst)
  ACT: sigmoid(i,f) psum->sbuf, relu(g) psum->sbuf@0, sigmoid(o) psum->psum
  DVE: m2 = i*g, c = m1+m2 (in place), h = c*o (mixed sbuf+psum)
  GP : m1 = f*c (all-sbuf, same base partition 32)
"""
import os
import sys

sys.path.insert(0, "/opt/trn_rl_repo")

import numpy as np

B, T, F, H = 2048, 512, 64, 16
NCORES = 8
BC = B // NCORES            # 256 batch per core
NCHAIN = 2                  # independent chains per core
CB = BC // NCHAIN           # 128 batch per chain
G = 2                       # groups packed on partitions per chain
BG = CB // G                # 64 batch per group (= free dim of most ops)
PROJ_STEPS = 8              # steps per proj burst: 8 * BG = 512 = 1 psum bank
TCHUNK = 32                 # steps per x DMA chunk
M1_ENGINE = os.environ.get("LSTM_M1_ENGINE", "vector")   # 'gpsimd' | 'vector'
T_RUN = int(os.environ.get("LSTM_T", T))                 # debug: fewer steps

# Keras kernel column order is [i, f, g, o] blocks of H.
_KERAS_GATE = {0: 0, 1: 1, 2: 3, 3: 2}   # our block (i,f,o,g) -> keras block


def _unit_perm():
    """keras column index for each of our 128 output units."""
    cols = np.zeros(4 * H * G, dtype=np.int64)
    grp = np.zeros(4 * H * G, dtype=np.int64)
    for u in range(4 * H * G):
        blk = u // (H * G)              # 0=i 1=f 2=o 3=g
        within = u % (H * G)
        g = within // H
        hh = within % H
        cols[u] = _KERAS_GATE[blk] * H + hh
        grp[u] = g
    return cols, grp


def _pack_weights(Wx, Wh, b, W2, b2, W3, b3, Wo, bo):
    cols, grp = _unit_perm()
    U = 4 * H * G
    wh2 = np.zeros((H * G, U), dtype=np.float32)
    wx2 = np.zeros((F * G, U), dtype=np.float32)
    for u in range(U):
        gg = grp[u]
        wh2[H * gg:H * (gg + 1), u] = Wh[:, cols[u]]
        wx2[F * gg:F * (gg + 1), u] = Wx[:, cols[u]]
    ball = b[cols].astype(np.float32).reshape(U, 1)

    w2d = np.zeros((H * G, 8 * G), dtype=np.float32)
    w3d = np.zeros((8 * G, 4 * G), dtype=np.float32)
    wod = np.zeros((4 * G, G), dtype=np.float32)
    for g in range(G):
        w2d[H * g:H * (g + 1), 8 * g:8 * (g + 1)] = W2
        w3d[8 * g:8 * (g + 1), 4 * g:4 * (g + 1)] = W3
        wod[4 * g:4 * (g + 1), g:g + 1] = Wo
    b2d = np.tile(b2, G).astype(np.float32).reshape(8 * G, 1)
    b3d = np.tile(b3, G).astype(np.float32).reshape(4 * G, 1)
    bod = np.tile(bo, G).astype(np.float32).reshape(G, 1)
    return wh2, wx2, ball, w2d, b2d, w3d, b3d, wod, bod


def _build_module():
    from contextlib import ExitStack

    import concourse.bass as bass
    import concourse.tile as tile
    from concourse import mybir
    from concourse.tile_rust import add_dep_helper

    AF = mybir.ActivationFunctionType
    nc = bass.Bass("TRN2", debug=False)

    # DRAM tensors (per-core inputs supplied via in_maps)
    xs = [
        nc.dram_tensor(f"xs{c}", [F * G, T * BG], mybir.dt.float32,
                       kind="ExternalInput").ap()
        for c in range(NCHAIN)
    ]
    wh2_d = nc.dram_tensor("wh2", [H * G, 128], mybir.dt.float32,
                           kind="ExternalInput").ap()
    wx2_d = nc.dram_tensor("wx2", [F * G, 128], mybir.dt.float32,
                           kind="ExternalInput").ap()
    ball_d = nc.dram_tensor("ball", [128, 1], mybir.dt.float32,
                            kind="ExternalInput").ap()
    w2d_d = nc.dram_tensor("w2d", [H * G, 8 * G], mybir.dt.float32,
                           kind="ExternalInput").ap()
    b2d_d = nc.dram_tensor("b2d", [8 * G, 1], mybir.dt.float32,
                           kind="ExternalInput").ap()
    w3d_d = nc.dram_tensor("w3d", [8 * G, 4 * G], mybir.dt.float32,
                           kind="ExternalInput").ap()
    b3d_d = nc.dram_tensor("b3d", [4 * G, 1], mybir.dt.float32,
                           kind="ExternalInput").ap()
    wod_d = nc.dram_tensor("wod", [4 * G, G], mybir.dt.float32,
                           kind="ExternalInput").ap()
    bod_d = nc.dram_tensor("bod", [G, 1], mybir.dt.float32,
                           kind="ExternalInput").ap()
    y_d = nc.dram_tensor("y", [2 * NCHAIN, BG], mybir.dt.float32,
                         kind="ExternalOutput").ap()

    n_chunks = (T_RUN + TCHUNK - 1) // TCHUNK

    with tile.TileContext(nc) as tc, ExitStack() as ctx:
        # All tiles are STATIC (bufs=1, allocated once, rotated manually):
        # pool-slot releases inject un-strippable waits, and hardware
        # instructions only carry ONE sync-wait slot.
        const = ctx.enter_context(tc.tile_pool(name="const", bufs=1))
        state = ctx.enter_context(tc.tile_pool(name="state", bufs=1))
        pspool = ctx.enter_context(tc.tile_pool(name="ps", bufs=1, space="PSUM"))

        # --- wait-count management --------------------------------------
        all_insts = {}          # name -> BassInstruction

        def reg(bi):
            all_insts[bi.ins.name] = bi
            # Same-engine sync deps -> nosync ordering edges: engines
            # complete strictly in order, so program order suffices and the
            # single hardware wait slot stays free for cross-engine deps.
            eng = bi.ins.engine
            for nm in list(bi.ins.sync_dependency_names()):
                di = all_insts.get(nm)
                if di is not None and di.ins.engine == eng:
                    bi.ins.try_remove_dependency(nm)
                    add_dep_helper(bi.ins, di.ins, sync=False,
                                   reason="same-engine order")
            return bi

        def absorb(op, nop_maker, pred):
            """Move op's sync deps matching pred onto same-engine sequencer
            nops (one per dependency proc, each carrying a single-sem wait),
            ordered before op via nosync edges; drop those deps from op."""
            groups = {}
            for nm in list(op.ins.sync_dependency_names()):
                di = all_insts.get(nm)
                if di is not None and pred(di):
                    groups.setdefault(str(di.ins.engine), []).append((nm, di))
            if not groups:
                return False
            prev = None
            for _eng, deps in sorted(groups.items()):
                n = nop_maker()
                for nm, di in deps:
                    add_dep_helper(n.ins, di.ins, reason="absorbed wait")
                    op.ins.try_remove_dependency(nm)
                if prev is not None:
                    add_dep_helper(n.ins, prev.ins, sync=False,
                                   reason="chain wait nops")
                prev = n
            add_dep_helper(op.ins, prev.ins, sync=False,
                           reason="order after wait-absorbing nop")
            return True

        def is_act(bi):
            return bi.ins.engine == mybir.EngineType.Activation

        def is_dve(bi):
            return bi.ins.engine == mybir.EngineType.DVE

        def is_pe(bi):
            return bi.ins.engine == mybir.EngineType.PE

        def is_dve_init(bi):
            # fence copies / state memsets — safe to absorb off matmuls;
            # never matches the hmul data dependency (TensorTensor).
            return is_dve(bi) and bi.ins.opcode in ("TensorCopy", "Memset")

        tail_deps = []

        def load_const(name, shape, dram_ap):
            raw = const.tile(shape, mybir.dt.float32, name=f"{name}_raw")
            tail_deps.append(reg(nc.gpsimd.dma_start(raw[:], dram_ap[:])))
            t = const.tile(shape, mybir.dt.float32, name=name)
            reg(nc.vector.tensor_copy(t[:], raw[:]))
            return t

        wh2_t = load_const("wh2t", [H * G, 128], wh2_d)
        wx2_t = load_const("wx2t", [F * G, 128], wx2_d)
        ball_t = load_const("ballt", [128, 1], ball_d)
        w2d_t = load_const("w2dt", [H * G, 8 * G], w2d_d)
        b2d_t = load_const("b2dt", [8 * G, 1], b2d_d)
        w3d_t = load_const("w3dt", [8 * G, 4 * G], w3d_d)
        b3d_t = load_const("b3dt", [4 * G, 1], b3d_d)
        wod_t = load_const("wodt", [4 * G, G], wod_d)
        bod_t = load_const("bodt", [G, 1], bod_d)

        # persistent state per chain; c lives in rows 32:64 of a 64-row tile
        # so its base partition matches the f block of the sigmoid output.
        h_t, cfull_t, o_ps, sig_t, g_t, m1_t, m2_t = [], [], [], [], [], [], []
        zb_t, xt_t = [], []
        for c in range(NCHAIN):
            ht = state.tile([H * G, BG], mybir.dt.float32, name=f"h{c}")
            reg(nc.vector.memset(ht[:], 0.0))
            h_t.append(ht)
            cf = state.tile([2 * H * G, BG], mybir.dt.float32, name=f"c{c}")
            reg(nc.vector.memset(cf[32:64, :], 0.0))
            cfull_t.append(cf)
            o_ps.append(pspool.tile([128, 8 * BG], mybir.dt.float32,
                                    name=f"ob{c}"))
            sig_t.append(state.tile([2 * H * G, BG], mybir.dt.float32,
                                    name=f"sig{c}"))
            g_t.append(state.tile([H * G, BG], mybir.dt.float32,
                                  name=f"g{c}"))
            m1_t.append(state.tile([2 * H * G, BG], mybir.dt.float32,
                                   name=f"m1{c}"))
            m2_t.append(state.tile([2 * H * G, BG], mybir.dt.float32,
                                   name=f"m2{c}"))
            zb_t.append([pspool.tile([128, PROJ_STEPS * BG], mybir.dt.float32,
                                     name=f"zb{c}_{k}") for k in range(2)])
            xt_t.append([state.tile([F * G, TCHUNK * BG], mybir.dt.float32,
                                    name=f"xt{c}_{k}") for k in range(2)])

        m1_eng = nc.gpsimd if M1_ENGINE == "gpsimd" else nc.vector

        def dma_chunk(c, j):
            tl = xt_t[c][j % 2]
            d = reg(nc.sync.dma_start(
                tl[:], xs[c][:, j * TCHUNK * BG:(j + 1) * TCHUNK * BG]))
            # keep only the queue-serialization wait on the DMA itself;
            # the WAR dep on the proj matmuls reading the old chunk goes
            # through an SP sequencer nop.
            absorb(d, nc.sync.nop, is_pe)
            tail_deps.append(d)
            return tl

        for c in range(NCHAIN):
            dma_chunk(c, 0)

        for t in range(T_RUN):
            slot = t % PROJ_STEPS
            chunk = t // TCHUNK
            for c in range(NCHAIN):
                if t % TCHUNK == TCHUNK // 2 and chunk + 1 < n_chunks:
                    dma_chunk(c, chunk + 1)
                def _mm_fixup(mm):
                    # matmuls keep their DVE(hmul)/DMA wait; ACT deps (z-bank
                    # reader history) and init copies go through nops.
                    absorb(mm, nc.tensor.nop,
                           lambda di: is_act(di) or is_dve_init(di))

                if slot == 0:
                    # new proj burst into the alternate z bank
                    z = zb_t[c][(t // PROJ_STEPS) % 2]
                    zt_cur = z
                    off = (t % TCHUNK) * BG
                    mm_proj = reg(nc.tensor.matmul(
                        z[:], wx2_t[:],
                        xt_t[c][chunk % 2][:, off:off + PROJ_STEPS * BG],
                        start=True, stop=True,
                    ))
                    _mm_fixup(mm_proj)
                z = zb_t[c][(t // PROJ_STEPS) % 2]
                sl = slice(slot * BG, (slot + 1) * BG)
                # recurrent matmul accumulates into this step's slot
                mm_rec = reg(nc.tensor.matmul(
                    z[:, sl], wh2_t[:], h_t[c][:],
                    start=False, stop=True, skip_group_check=True,
                ))
                _mm_fixup(mm_rec)
                sig, g_sb, m1, m2 = sig_t[c], g_t[c], m1_t[c], m2_t[c]
                # ACT ops keep their PE wait; DVE-side deps (work-tile WAR,
                # o-slot WAR) go through ACT sequencer nops.
                # i,f sigmoid (+bias), psum -> sbuf rows 0:64
                a_sig1 = reg(nc.scalar.activation(sig[0:64, :], z[0:64, sl],
                                                  AF.Sigmoid,
                                                  bias=ball_t[0:64, :]))
                absorb(a_sig1, nc.scalar.nop, is_dve)
                # g relu, psum rows 96:128 -> sbuf rows 0:32
                a_relu = reg(nc.scalar.activation(g_sb[:], z[96:128, sl],
                                                  AF.Relu,
                                                  bias=ball_t[96:128, :]))
                absorb(a_relu, nc.scalar.nop, is_dve)
                # o sigmoid, psum rows 64:96 -> o-bank psum rows 64:96
                osl = slice((t % 8) * BG, (t % 8 + 1) * BG)
                a_sig2 = reg(nc.scalar.activation(
                    o_ps[c][64:96, osl], z[64:96, sl],
                    AF.Sigmoid, bias=ball_t[64:96, :]))
                absorb(a_sig2, nc.scalar.nop, is_dve)
                # m2 = i * g  (both sbuf base 0) -> rows 32:64
                reg(nc.vector.tensor_mul(m2[32:64, :], sig[0:32, :], g_sb[:]))
                # m1 = f * c  (all sbuf base 32)
                reg(m1_eng.tensor_mul(m1[32:64, :], sig[32:64, :],
                                      cfull_t[c][32:64, :]))
                # c = m1 + m2 (in place)
                reg(nc.vector.tensor_add(cfull_t[c][32:64, :], m1[32:64, :],
                                         m2[32:64, :]))
                # h = c * o  (sbuf base 32 x psum base 64 -> sbuf base 0).
                # Its WAR dep on rec's h read is transitively covered via
                # sig2(t) <- rec(t); drop PE deps outright.
                hm = reg(nc.vector.tensor_mul(h_t[c][:], cfull_t[c][32:64, :],
                                              o_ps[c][64:96, osl]))
                for nm in list(hm.ins.sync_dependency_names()):
                    di = all_insts.get(nm)
                    if di is not None and is_pe(di):
                        hm.ins.try_remove_dependency(nm)

        # dense head per chain
        hp = pspool.tile([128, 4 * BG], mybir.dt.float32, name="headp")
        for c in range(NCHAIN):
            mmh1 = reg(nc.tensor.matmul(hp[0:8 * G, c * 2 * BG:c * 2 * BG + BG],
                                        w2d_t[:], h_t[c][:],
                                        start=True, stop=True))
            absorb(mmh1, nc.tensor.nop, lambda di: is_act(di) or is_dve_init(di))
            x2 = state.tile([8 * G, BG], mybir.dt.float32, name=f"x2{c}")
            a_x2 = reg(nc.scalar.activation(
                x2[:], hp[0:8 * G, c * 2 * BG:c * 2 * BG + BG],
                AF.Relu, bias=b2d_t[:]))
            absorb(a_x2, nc.scalar.nop, is_dve)
            mmh2 = reg(nc.tensor.matmul(
                hp[0:4 * G, c * 2 * BG + BG:c * 2 * BG + 2 * BG], w3d_t[:],
                x2[:], start=True, stop=True))
            absorb(mmh2, nc.tensor.nop, lambda di: is_act(di) or is_dve_init(di))
            x3 = state.tile([4 * G, BG], mybir.dt.float32, name=f"x3{c}")
            a_x3 = reg(nc.scalar.activation(
                x3[:], hp[0:4 * G, c * 2 * BG + BG:c * 2 * BG + 2 * BG],
                AF.Relu, bias=b3d_t[:]))
            absorb(a_x3, nc.scalar.nop, is_dve)
            ozp = o_ps[c]
            mmh3 = reg(nc.tensor.matmul(ozp[0:G, 0:BG], wod_t[:], x3[:],
                                        start=True, stop=True))
            absorb(mmh3, nc.tensor.nop,
                   lambda di: is_act(di) or is_dve(di))
            ych = state.tile([G, BG], mybir.dt.float32, name=f"y{c}")
            a_y = reg(nc.scalar.activation(ych[:], ozp[0:G, 0:BG], AF.Sigmoid,
                                           bias=bod_t[:]))
            absorb(a_y, nc.scalar.nop, is_dve)
            dma_y = reg(nc.gpsimd.dma_start(y_d[G * c:G * (c + 1), :], ych[:]))
            absorb(dma_y, nc.gpsimd.nop, is_act)
            tail_deps.append(mmh3)
            tail_deps.append(dma_y)
            tail_deps.append(a_y)

        # Tail: single-wait SP nops make the SP sequencer observe every
        # proc's final tick so the framework's kernel-tail Drain (one wait
        # slot) needs no further waits.
        prev = None
        for bi in tail_deps:
            n = nc.sync.nop()
            add_dep_helper(n.ins, bi.ins, reason="tail observe")
            if prev is not None:
                add_dep_helper(n.ins, prev.ins, sync=False,
                               reason="tail chain")
            prev = n

    return nc


_CACHE = {}


def _prep_x(x):
    """Per (core, chain): [F*G, T*BG] with rows 0:F = group0 (features-major),
    rows F:2F = group1; free layout (t, b)."""
    out = []
    for core in range(NCORES):
        chains = []
        for ch in range(NCHAIN):
            base = core * BC + ch * CB
            g0 = x[base:base + BG]                 # [BG, T, F]
            g1 = x[base + BG:base + 2 * BG]
            s0 = np.ascontiguousarray(g0.transpose(2, 1, 0)).reshape(F, T * BG)
            s1 = np.ascontiguousarray(g1.transpose(2, 1, 0)).reshape(F, T * BG)
            chains.append(np.concatenate([s0, s1], axis=0))
        out.append(chains)
    return out


def kernel(x, Wx, Wh, b, W2, b2, W3, b3, Wo, bo, _trace=False):
    from concourse import bass_utils

    x = np.asarray(x, dtype=np.float32)
    wh2, wx2, ball, w2d, b2d, w3d, b3d, wod, bod = _pack_weights(
        np.asarray(Wx, np.float32), np.asarray(Wh, np.float32),
        np.asarray(b, np.float32), np.asarray(W2, np.float32),
        np.asarray(b2, np.float32), np.asarray(W3, np.float32),
        np.asarray(b3, np.float32), np.asarray(Wo, np.float32),
        np.asarray(bo, np.float32))

    if "nc" not in _CACHE:
        _CACHE["nc"] = _build_module()
    nc = _CACHE["nc"]

    xprep = _prep_x(x)
    in_maps = []
    for core in range(NCORES):
        m = {
            "wh2": wh2, "wx2": wx2, "ball": ball,
            "w2d": w2d, "b2d": b2d, "w3d": w3d, "b3d": b3d,
            "wod": wod, "bod": bod,
        }
        for ch in range(NCHAIN):
            m[f"xs{ch}"] = xprep[core][ch]
        in_maps.append(m)

    res = bass_utils.run_bass_kernel_spmd(
        nc, in_maps, core_ids=list(range(NCORES)), trace=_trace,
    )
    ys = []
    for core in range(NCORES):
        y = res.results[core]["y"]          # [2*NCHAIN, BG]
        ys.append(y.reshape(BC))
    out = np.concatenate(ys).reshape(B, 1).astype(np.float32)
    if _trace:
        out = (out, res)
    return out


def bench(inputs, iters=20):
    """Median wall time of repeated device executes with device-resident
    inputs (upper bound on per-launch HW time; includes dispatch overhead)."""
    import time

    import jax
    import numpy as np_
    from jax.sharding import Mesh, NamedSharding, PartitionSpec

    from concourse import bass2jax, mybir

    if "nc" not in _CACHE:
        _CACHE["nc"] = _build_module()
    nc = _CACHE["nc"]

    x = np.asarray(inputs["x"], dtype=np.float32)
    wh2, wx2, ball, w2d, b2d, w3d, b3d, wod, bod = _pack_weights(
        *[np.asarray(inputs[k], np.float32) for k in
          ("Wx", "Wh", "b", "W2", "b2", "W3", "b3", "Wo", "bo")])
    xprep = _prep_x(x)
    in_maps = []
    for core in range(NCORES):
        m = {"wh2": wh2, "wx2": wx2, "ball": ball, "w2d": w2d, "b2d": b2d,
             "w3d": w3d, "b3d": b3d, "wod": wod, "bod": bod}
        for ch in range(NCHAIN):
            m[f"xs{ch}"] = xprep[core][ch]
        in_maps.append(m)

    bass2jax.install_neuronx_cc_hook()
    from jax.experimental.shard_map import shard_map

    partition_name = (nc.partition_id_tensor.name
                      if nc.partition_id_tensor else None)
    in_names, out_names, out_avals, zero_outs = [], [], [], []
    for alloc in nc.m.functions[0].allocations:
        if not isinstance(alloc, mybir.MemoryLocationSet):
            continue
        name = alloc.memorylocations[0].name
        if alloc.kind == "ExternalInput":
            if name != partition_name:
                in_names.append(name)
        elif alloc.kind == "ExternalOutput":
            out_names.append(name)
            shape = tuple(alloc.tensor_shape)
            np_dt = mybir.dt.np(alloc.dtype)
            out_avals.append(jax.core.ShapedArray(shape, np_dt))
            zero_outs.append(np_.zeros(shape, np_dt))
    n_params = len(in_names)
    n_outs = len(out_avals)
    all_names = in_names + out_names
    if partition_name is not None:
        all_names = all_names + [partition_name]

    def _body(*args):
        operands = list(args)
        if partition_name is not None:
            operands.append(bass2jax.partition_id_tensor())
        outs = bass2jax._bass_exec_p.bind(
            *operands,
            out_avals=tuple(out_avals),
            in_names=tuple(all_names),
            out_names=tuple(out_names),
            lowering_input_output_aliases=(),
            sim_require_finite=True,
            sim_require_nnan=True,
            nc=nc,
        )
        return tuple(outs)

    devices = jax.devices()[:NCORES]
    mesh = Mesh(np_.asarray(devices), ("core",))
    in_specs = (PartitionSpec("core"),) * (n_params + n_outs)
    out_specs = (PartitionSpec("core"),) * n_outs
    sharded = jax.jit(
        shard_map(_body, mesh=mesh, in_specs=in_specs, out_specs=out_specs,
                  check_rep=False),
        keep_unused=True,
    )
    shard = NamedSharding(mesh, PartitionSpec("core"))
    concat_in = [
        jax.device_put(
            np_.concatenate([np_.asarray(in_maps[c][nm]) for c in range(NCORES)],
                            axis=0), shard)
        for nm in in_names[:n_params]
    ]
    concat_zeros = [
        jax.device_put(np_.zeros((NCORES * z.shape[0], *z.shape[1:]), z.dtype),
                       shard)
        for z in zero_outs
    ]
    # warmup (compile)
    out = sharded(*concat_in, *concat_zeros)
    jax.block_until_ready(out)
    times = []
    for _ in range(iters):
        t0 = time.perf_counter()
        out = sharded(*concat_in, *concat_zeros)
        jax.block_until_ready(out)
        times.append(time.perf_counter() - t0)
    times.sort()
    return times[len(times) // 4] * 1e9   # lower-quartile wall ns
